# revision 29
# baseline (speedup 1.0000x reference)
"""Trainium2 Bass kernel for nn_CAModel (neural cellular automaton step).

Computation (per image, fp32):
  pre_life = maxpool3x3(x[...,3]) > 0.1        (HOST: exact fp32)
  gx, gy   = depthwise 3x3 sobel convs of x
  perc     = interleave([x, gx, gy])            # [H,W,48]
  h        = relu(perc @ w0)                    # [H,W,128]
  dx       = h @ w1                             # [H,W,16]
  x_mid    = x + dx * (update_rand <= 0.5)      (update mask from HOST)
  life     = pre_life & (maxpool3x3(x_mid[...,3]) > 0.1)
  x_new    = x_mid * life
  returns (x_new, dx)

Mapping: 8 NeuronCores, data-parallel over batch (2 images/core).

Device pipeline per 16-row tile (2 psum-halves q of 4 row-pair groups):
  fc0: sobel folded into weights; two K-stacked matmuls per row-pair
       (taps dx=0,1 stacked on 96 partitions, tap dx=2 separate) instead
       of three -> 2 streamed columns/pixel on the PE.
  relu: split across ACT and DVE engines (psum -> bf16 sbuf).
  fc1: w1-stationary (16-col weight loads), 4 row-pair groups packed
       into one PSUM tile at col-group offsets 0/32/64/96.
  dx transpose: channel-major [16,512] slabs -> pixel-major via HWDGE
       xbar DMA transpose (bf16).
  tail (gpsimd): x_mid = x + dx*um; alpha extract; per-image maxpool
       (vertical on DVE, horizontal via bf16 shift matmuls on PE),
       life mult, x_new store.  All I/O in bf16.
"""

import functools
import os
import sys

import numpy as np

_TRN_REPO = os.environ.get("TRN_RL_REPO", "/opt/trn_rl_repo")
if _TRN_REPO not in sys.path:
    sys.path.insert(0, _TRN_REPO)

import concourse.bass as bass
import concourse.bacc as bacc
import concourse.tile as tile
from concourse import mybir
from concourse.bass_utils import run_bass_kernel_spmd

F32 = mybir.dt.float32
BF16 = mybir.dt.bfloat16
BF16_NP = mybir.dt.np(mybir.dt.bfloat16)

C = 16          # channels
HID = 128       # hidden dim
PW = 128        # partitions used as w-position within a half
N_CORES = 8
FIRE_RATE = 0.5
ALIVE_THR = 0.1

LAST_RESULTS = None  # BassKernelResults of the most recent kernel() call


# ---------------------------------------------------------------------------
# device program
# ---------------------------------------------------------------------------

def build_program(NI, H, W, TR=16):
    """Build the Bass program for one core processing NI images of HxW."""
    NH = W // PW                  # halves per row (2)
    assert W % PW == 0 and H % TR == 0 and TR == 16 and NH == 2
    Hp, Wp = H + 2, W + 2
    NRH = NI * H * NH             # total (img,row,half) count
    NT = NI * H // TR             # tiles

    nc = bacc.Bacc(trn_type="TRN2")

    xch = nc.dram_tensor("xch", [NI * C * Hp + 1, Wp], BF16, kind="ExternalInput")
    xpx_d = nc.dram_tensor("xpx", [PW, NRH, C], BF16, kind="ExternalInput")
    um_d = nc.dram_tensor("um", [PW, NRH], BF16, kind="ExternalInput")
    plx_d = nc.dram_tensor("plx", [PW, NRH], BF16, kind="ExternalInput")
    b01_d = nc.dram_tensor("b01", [96, HID], BF16, kind="ExternalInput")
    b2_d = nc.dram_tensor("b2", [48, HID], BF16, kind="ExternalInput")
    w1_d = nc.dram_tensor("w1d", [HID, C], BF16, kind="ExternalInput")
    se_d = nc.dram_tensor("SEd", [PW, PW], BF16, kind="ExternalInput")
    sw_d = nc.dram_tensor("SWd", [PW, PW], BF16, kind="ExternalInput")
    # dx stored pixel-major: [st, ti, w(128), rh(32), c]
    dxo = nc.dram_tensor("dxo", [NT // 2, 2, PW, 2 * TR, C], BF16,
                         kind="ExternalOutput")
    xno = nc.dram_tensor("xno", [PW, NRH, C], BF16, kind="ExternalOutput")

    with tile.TileContext(nc) as tc:
        _emit(tc, locals())
    nc.compile()
    return nc


def _emit(tc, t):
    nc = tc.nc
    NI, H, W, TR = t["NI"], t["H"], t["W"], t["TR"]
    NH, Hp, Wp, NRH, NT = t["NH"], t["Hp"], t["Wp"], t["NRH"], t["NT"]
    xch, xpx_d, um_d, plx_d = t["xch"], t["xpx_d"], t["um_d"], t["plx_d"]
    b01_d, b2_d, w1_d, se_d, sw_d = (
        t["b01_d"], t["b2_d"], t["w1_d"], t["se_d"], t["sw_d"])
    dxo, xno = t["dxo"], t["xno"]
    HNH = H * NH                  # per-image rh span (512)
    AL = mybir.AluOpType
    Relu = mybir.ActivationFunctionType.Relu

    from contextlib import ExitStack
    ctx = ExitStack()
    with ctx:
        singles = ctx.enter_context(tc.tile_pool(name="singles", bufs=1))
        xc_pool = ctx.enter_context(tc.tile_pool(name="xc", bufs=8))
        hs_pool = ctx.enter_context(tc.tile_pool(name="hs", bufs=10))
        xp_pool = ctx.enter_context(tc.tile_pool(name="xp", bufs=4))
        dxs_pool = ctx.enter_context(tc.tile_pool(name="dxs", bufs=4))
        dxm_pool = ctx.enter_context(tc.tile_pool(name="dxm", bufs=4))
        xns_pool = ctx.enter_context(tc.tile_pool(name="xns", bufs=3))
        ps_h = ctx.enter_context(tc.tile_pool(name="ps_h", bufs=6, space="PSUM"))
        ps_dx = ctx.enter_context(tc.tile_pool(name="ps_dx", bufs=2, space="PSUM"))

        # ---- constants / weights / masks (gpsimd queue: SWDGE descriptors
        # go out immediately, ahead of the bulk sync-queue traffic) ----
        b01_sb = singles.tile([96, HID], BF16)
        nc.gpsimd.dma_start(out=b01_sb, in_=b01_d.ap())
        b2_sb = singles.tile([48, HID], BF16)
        nc.gpsimd.dma_start(out=b2_sb, in_=b2_d.ap())
        w1_sb = singles.tile([HID, C], BF16)
        nc.gpsimd.dma_start(out=w1_sb, in_=w1_d.ap())
        se_sb = singles.tile([PW, PW], BF16)
        nc.sync.dma_start(out=se_sb, in_=se_d.ap())
        sw_sb = singles.tile([PW, PW], BF16)
        nc.sync.dma_start(out=sw_sb, in_=sw_d.ap())

        um_sb = singles.tile([PW, NRH], BF16)
        plx_sb = singles.tile([PW, NRH], BF16)

        # ---- residents / mask scratch ----
        # xmid padded with NH zero-rows per image on both sides so the
        # vertical 3-max needs no edge-case ops (alpha 0 < 0.1 threshold)
        IMS = HNH + 2 * NH            # per-image rh stride in xmidp
        xmidp = singles.tile([PW, NI * IMS, C], BF16)
        for img in range(NI):
            nc.vector.memset(xmidp[:, img * IMS:img * IMS + NH, :], 0.0)
            nc.vector.memset(
                xmidp[:, img * IMS + NH + HNH:(img + 1) * IMS, :], 0.0)

        def xmid(rh0, n):
            # rh0 in unpadded global coords -> padded view [PW, n, C]
            img = rh0 // HNH
            base = img * IMS + NH + (rh0 - img * HNH)
            return xmidp[:, base:base + n, :]
        vm = singles.tile([PW, HNH], BF16)
        vm2 = singles.tile([PW, HNH], BF16)
        m3 = singles.tile([PW, HNH], BF16)
        life = singles.tile([PW, HNH], BF16)
        seam = singles.tile([PW, 2 * H], BF16)
        nc.vector.memset(seam, 0.0)

        # ---- PE pre-sync dummies (touch the hot matmul operands once) ----
        scr = ps_dx.tile([PW, 2], F32, tag="dxc")
        nc.tensor.matmul(out=scr, lhsT=b01_sb, rhs=b01_sb[:, 0:2],
                         start=True, stop=True)
        nc.tensor.matmul(out=scr, lhsT=b2_sb, rhs=b2_sb[:, 0:2],
                         start=True, stop=True)

        def half_slice(tile_, p0, cnt, hf, r0, n):
            return tile_[p0:p0 + cnt, r0:r0 + n].rearrange(
                "p (r h) -> p r h", h=NH)[:, :, hf]

        def mask_chunk(img, ck):
            # maxpool/life/x_new for rows [32*ck, 32*ck+32) of image img
            S = img * HNH
            B = img * IMS + NH
            r0 = ck * 64                  # rh offset within image
            CH = 64

            def A(k):  # alpha of x_mid rows, padded, strided view
                return xmidp[:, B + r0 + k:B + r0 + k + CH, 3]

            # vertical 3-max over zero-padded rows
            nc.vector.tensor_tensor(
                out=vm[:, r0:r0 + CH], in0=A(0), in1=A(-NH), op=AL.max)
            nc.vector.tensor_tensor(
                out=vm2[:, r0:r0 + CH], in0=vm[:, r0:r0 + CH], in1=A(NH),
                op=AL.max)

            # horizontal 3-max: PE shift-permutation matmuls (bf16) + seams
            pse = ps_h.tile([PW, CH], F32, tag="psh")
            nc.tensor.matmul(out=pse, lhsT=se_sb, rhs=vm2[:, r0:r0 + CH],
                             start=True, stop=True)
            psw = ps_h.tile([PW, CH], F32, tag="psh")
            nc.tensor.matmul(out=psw, lhsT=sw_sb, rhs=vm2[:, r0:r0 + CH],
                             start=True, stop=True)
            nc.vector.tensor_tensor(out=m3[:, r0:r0 + CH],
                                    in0=vm2[:, r0:r0 + CH], in1=pse, op=AL.max)
            nc.vector.tensor_tensor(out=m3[:, r0:r0 + CH],
                                    in0=m3[:, r0:r0 + CH], in1=psw, op=AL.max)

            # seam fixes between the two w-halves (rows 32*ck..32*ck+31)
            c0 = ck * 32
            nc.sync.dma_start(
                out=seam[127:128, c0:c0 + 32],
                in_=vm2[0:1, r0:r0 + CH].rearrange(
                    "p (r h) -> p r h", h=NH)[:, :, 1])
            nc.vector.tensor_tensor(
                out=half_slice(m3, 96, 32, 0, r0, CH),
                in0=half_slice(m3, 96, 32, 0, r0, CH),
                in1=seam[96:128, c0:c0 + 32], op=AL.max)
            nc.sync.dma_start(
                out=seam[0:1, H + c0:H + c0 + 32],
                in_=vm2[127:128, r0:r0 + CH].rearrange(
                    "p (r h) -> p r h", h=NH)[:, :, 0])
            nc.vector.tensor_tensor(
                out=half_slice(m3, 0, 32, 1, r0, CH),
                in0=half_slice(m3, 0, 32, 1, r0, CH),
                in1=seam[0:32, H + c0:H + c0 + 32], op=AL.max)

            # life = (m3 > thr) * pre_life
            nc.vector.scalar_tensor_tensor(
                out=life[:, r0:r0 + CH], in0=m3[:, r0:r0 + CH],
                scalar=ALIVE_THR, in1=plx_sb[:, S + r0:S + r0 + CH],
                op0=AL.is_gt, op1=AL.mult)

            # x_new = x_mid * life
            xns = xns_pool.tile([PW, CH, C], BF16)
            nc.gpsimd.tensor_tensor(
                out=xns, in0=xmid(S + r0, CH),
                in1=life[:, r0:r0 + CH, None].to_broadcast([PW, CH, C]),
                op=AL.mult)
            nc.sync.dma_start(out=xno.ap()[:, S + r0:S + r0 + CH, :], in_=xns)

        # ================= main loop (2-tile super-tiles) =================
        NST = NT // 2

        def emit_loads(st):
            rh_s = st * 2 * TR * NH
            xp = xp_pool.tile([PW, 2 * TR * NH, C], BF16)
            nc.sync.dma_start(
                out=xp, in_=xpx_d.ap()[:, rh_s:rh_s + 2 * TR * NH, :])
            xcs = []
            for ti in range(2):
                tt = 2 * st + ti
                img, t_in = divmod(tt, H // TR)
                a = t_in * TR
                # x in conv layout: partitions (s,dy,c); partitions 48..95
                # hold the same rows shifted one column (tap stacking).
                # SWDGE: its partition-interleave swizzle spreads the big
                # per-partition descriptors across all 16 SDMA engines,
                # where HWDGE piles them onto 3.
                xc2 = xc_pool.tile([96, TR, Wp], BF16)
                base = (img * C * Hp + a) * Wp
                for s in range(2):
                    # rows are contiguous in DRAM and in the SBUF tile, so
                    # one TR*Wp run per partition (48 fat descriptors, not
                    # 768 thin ones)
                    sap = bass.AP(
                        tensor=xch.ap().tensor, offset=base + s,
                        ap=[[Wp, 3], [Hp * Wp, C], [1, TR * Wp]])
                    nc.gpsimd.dma_start(
                        out=xc2[48 * s:48 * s + 48].rearrange(
                            "p r w -> p (r w)"),
                        in_=sap)
                xcs.append(xc2)
            return xp, xcs

        lds = [emit_loads(0), emit_loads(1), emit_loads(2)]
        nc.sync.dma_start(out=um_sb, in_=um_d.ap())
        nc.sync.dma_start(out=plx_sb, in_=plx_d.ap())
        for st in range(NST):
            xp, xcs = lds[st % 3]
            rh_s = st * 2 * TR * NH
            for ti in range(2):
                xc2 = xcs[ti]

                # ---- fc0 + fc1 woven per q so the Tensor FIFO never has a
                # long stretch of LDW-heavy 16-col fc1 matmuls (keeps the
                # HAM activity monitor seeing MM streaming in every window).
                # fc1 is h-STATIONARY (out = h_block.T @ w1): dx lands
                # pixel-major [128w, rh, C] directly in PSUM -> no garbage
                # lanes and no channel->pixel transpose anywhere. ----
                pshs = [None] * 8
                hsbs = [None] * 8
                pdpx = ps_dx.tile([PW, 2 * TR, C], F32, tag="dxc")
                for q in range(2):
                    for g in range(4):
                        pp = q * 4 + g
                        psh = ps_h.tile([HID, 512], F32, tag="psh")
                        pshs[pp] = psh
                        nc.tensor.matmul(
                            out=psh, lhsT=b01_sb,
                            rhs=xc2[0:96, 2 * pp:2 * pp + 2, 0:W],
                            start=True, stop=False, skip_group_check=True)
                    for g in range(4):
                        pp = q * 4 + g
                        nc.tensor.matmul(
                            out=pshs[pp], lhsT=b2_sb,
                            rhs=xc2[0:48, 2 * pp:2 * pp + 2, 2:2 + W],
                            start=False, stop=True, skip_group_check=True)
                    for g in range(4):
                        pp = q * 4 + g
                        hsb = hs_pool.tile([HID, 512], BF16)
                        hsbs[pp] = hsb
                        if pp % 8 not in (3, 5, 7):
                            nc.scalar.activation(
                                out=hsb, in_=pshs[pp], func=Relu)
                        else:
                            nc.vector.tensor_scalar(
                                out=hsb, in0=pshs[pp], scalar1=0.0,
                                scalar2=None, op0=AL.max)
                    for g in range(4):
                        pp = q * 4 + g
                        for b in range(4):
                            # hsb col block b=(r2,h): rh index = 4*pp+2*r2+h
                            nc.tensor.matmul(
                                out=pdpx[:, 4 * pp + b, :],
                                lhsT=hsbs[pp][:, 128 * b:128 * b + 128],
                                rhs=w1_sb, start=True, stop=True,
                                skip_group_check=True)

                # drain dx to SBUF (bf16, for the dx output) and apply the
                # update mask straight out of PSUM (DVE) for x_mid
                rh0 = rh_s + ti * 2 * TR
                umv = um_sb[:, rh0:rh0 + 2 * TR][:, :, None].to_broadcast(
                    [PW, 2 * TR, C])
                dxs = dxs_pool.tile([PW, 2 * TR, C], BF16)
                dxm = dxm_pool.tile([PW, 2 * TR, C], F32)
                nc.scalar.copy(out=dxs, in_=pdpx)
                nc.vector.tensor_tensor(out=dxm, in0=pdpx, in1=umv,
                                        op=AL.mult)
                nc.sync.dma_start(out=dxo.ap()[st, ti], in_=dxs)
                nc.gpsimd.tensor_tensor(
                    out=xmid(rh0, 2 * TR),
                    in0=xp[:, ti * 2 * TR:(ti + 1) * 2 * TR, :], in1=dxm,
                    op=AL.add)

            # prefetch three super-tiles ahead so the loads complete well
            # before fc0 needs them (the issue queue runs behind the tail)
            if st + 3 < NST:
                lds[st % 3] = emit_loads(st + 3)

            # mask chunks lag TWO super-tiles: chunk gc needs x_mid rows
            # through the first row of chunk gc+1 (super-tile gc+1), whose
            # tail ran during iteration gc+1.  Emitting at gc+2 means the
            # pse/psw shift-matmuls never head-of-line-block the Tensor
            # FIFO waiting on the transpose->gpsimd tail chain.
            if st >= 2:
                mask_chunk(*divmod(st - 2, NST // NI))

        # drain the last two mask chunks
        for gc in range(max(NST - 2, 0), NST):
            mask_chunk(*divmod(gc, NST // NI))


# ---------------------------------------------------------------------------
# host side
# ---------------------------------------------------------------------------

def _sobel():
    kx = np.outer([1.0, 2.0, 1.0], [-1.0, 0.0, 1.0]) / 8.0
    ky = kx.T
    return kx, ky


def make_weights(w0, w1):
    """Fold sobel taps into fc0 -> b01 [96,128] (taps dx=0,1), b2 [48,128]."""
    kx, ky = _sobel()
    w0 = np.asarray(w0, np.float32)          # [48, 128]
    W0x = w0[0::3]                           # [16, 128]
    W0gx = w0[1::3]
    W0gy = w0[2::3]
    Bw = np.zeros((3, 48, HID), np.float32)
    for dy in range(3):
        for dxi in range(3):
            m = kx[dy, dxi] * W0gx + ky[dy, dxi] * W0gy
            if dy == 1 and dxi == 1:
                m = m + W0x
            Bw[dxi, dy * C:(dy + 1) * C, :] = m
    b01 = np.concatenate([Bw[0], Bw[1]], axis=0).astype(BF16_NP)
    b2 = Bw[2].astype(BF16_NP)
    return b01, b2, np.asarray(w1, BF16_NP)


def _maxpool3(a):
    """3x3 max pool, stride 1, -inf padding. a: [NI, H, W]."""
    p = np.pad(a, ((0, 0), (1, 1), (1, 1)), constant_values=-np.inf)
    v = np.maximum(np.maximum(p[:, :-2, :], p[:, 1:-1, :]), p[:, 2:, :])
    return np.maximum(np.maximum(v[:, :, :-2], v[:, :, 1:-1]), v[:, :, 2:])


def _to_rh(a, NI, H, NH):
    """[NI, H, W] -> [PW, NI*H*NH] pixel-slab layout."""
    return np.ascontiguousarray(
        a.reshape(NI, H, NH, PW).transpose(3, 0, 1, 2)).reshape(PW, NI * H * NH)


def host_inputs(x_core, ur_core, b01, b2, w1, H, W):
    """Build the per-core input map from [NI,H,W,C] x and [NI,H,W,1] rand."""
    NI = x_core.shape[0]
    NH = W // PW
    Hp, Wp = H + 2, W + 2
    NRH = NI * H * NH

    xch = np.zeros((NI, C, Hp, Wp), BF16_NP)
    xch[:, :, 1:H + 1, 1:W + 1] = x_core.transpose(0, 3, 1, 2)
    xch = np.concatenate(
        [xch.reshape(NI * C * Hp, Wp), np.zeros((1, Wp), BF16_NP)], axis=0)

    x_px = np.ascontiguousarray(
        x_core.reshape(NI, H, NH, PW, C).transpose(3, 0, 1, 2, 4)
    ).reshape(PW, NRH, C).astype(BF16_NP)

    um = (ur_core[..., 0] <= FIRE_RATE).astype(np.float32)
    um_p = _to_rh(um, NI, H, NH).astype(BF16_NP)

    pre = (_maxpool3(x_core[..., 3]) > ALIVE_THR).astype(np.float32)
    plx_p = _to_rh(pre, NI, H, NH).astype(BF16_NP)

    return {
        "xch": xch,
        "xpx": x_px,
        "um": um_p,
        "plx": plx_p,
        "b01": b01,
        "b2": b2,
        "w1d": w1,
        "SEd": np.eye(PW, k=-1, dtype=np.float32).astype(BF16_NP),
        "SWd": np.eye(PW, k=1, dtype=np.float32).astype(BF16_NP),
    }


def unpack_xno(dev, NI, H, W):
    """[PW, NRH, C] device layout -> [NI, H, W, C] float32."""
    NH = W // PW
    return np.ascontiguousarray(
        dev.astype(np.float32).reshape(PW, NI, H, NH, C).transpose(1, 2, 3, 0, 4)
    ).reshape(NI, H, W, C)


def unpack_dxo(dev, NI, H, W, TR=16):
    """[NST, 2, PW, 2*TR, C] pixel-major dx -> [NI, H, W, C] float32."""
    NSI = H // TR // 2            # super-tiles per image
    NH = W // PW
    # [img, st, ti, w, row, h, c] -> [img, st, ti, row, h, w, c]
    d = dev.astype(np.float32).reshape(NI, NSI, 2, PW, TR, NH, C)
    d = d.transpose(0, 1, 2, 4, 5, 3, 6)
    return np.ascontiguousarray(d).reshape(NI, H, W, C)


@functools.lru_cache(maxsize=2)
def _cached_program(NI, H, W, TR):
    return build_program(NI, H, W, TR=TR)


def kernel(x, update_rand, w0, w1):
    x = np.asarray(x, np.float32)
    update_rand = np.asarray(update_rand, np.float32)
    B, H, W, _ = x.shape
    NI = B // N_CORES
    b01, b2, w1f = make_weights(w0, w1)

    nc = _cached_program(NI, H, W, 16)
    in_maps = [
        host_inputs(x[i * NI:(i + 1) * NI], update_rand[i * NI:(i + 1) * NI],
                    b01, b2, w1f, H, W)
        for i in range(N_CORES)
    ]
    res = run_bass_kernel_spmd(nc, in_maps, core_ids=list(range(N_CORES)))
    global LAST_RESULTS
    LAST_RESULTS = res
    x_new = np.concatenate(
        [unpack_xno(r["xno"], NI, H, W) for r in res.results], axis=0)
    dx = np.concatenate(
        [unpack_dxo(r["dxo"], NI, H, W) for r in res.results], axis=0)
    return x_new, dx



# revision 41
# speedup vs baseline: 2.0943x; 2.0943x over previous
"""Trainium2 Bass kernel for nn_CAModel (neural cellular automaton step).

Computation (per image, fp32):
  pre_life = maxpool3x3(x[...,3]) > 0.1        (HOST: exact fp32)
  gx, gy   = depthwise 3x3 sobel convs of x
  perc     = interleave([x, gx, gy])            # [H,W,48]
  h        = relu(perc @ w0)                    # [H,W,128]
  dx       = h @ w1                             # [H,W,16]
  x_mid    = x + dx * (update_rand <= 0.5)      (update mask from HOST)
  life     = pre_life & (maxpool3x3(x_mid[...,3]) > 0.1)
  x_new    = x_mid * life
  returns (x_new, dx)

Mapping: 8 NeuronCores, data-parallel over batch (2 images/core).

Device pipeline per 16-row tile (2 psum-halves q of 4 row-pair groups):
  fc0: sobel folded into weights; two K-stacked matmuls per row-pair
       (taps dx=0,1 stacked on 96 partitions, tap dx=2 separate) instead
       of three -> 2 streamed columns/pixel on the PE.
  relu: split across ACT and DVE engines (psum -> bf16 sbuf).
  fc1: w1-stationary (16-col weight loads), 4 row-pair groups packed
       into one PSUM tile at col-group offsets 0/32/64/96.
  dx transpose: channel-major [16,512] slabs -> pixel-major via HWDGE
       xbar DMA transpose (bf16).
  tail (gpsimd): x_mid = x + dx*um; alpha extract; per-image maxpool
       (vertical on DVE, horizontal via bf16 shift matmuls on PE),
       life mult, x_new store.  All I/O in bf16.
"""

import functools
import os
import sys

import numpy as np

_TRN_REPO = os.environ.get("TRN_RL_REPO", "/opt/trn_rl_repo")
if _TRN_REPO not in sys.path:
    sys.path.insert(0, _TRN_REPO)

import concourse.bass as bass
import concourse.bacc as bacc
import concourse.tile as tile
from concourse import mybir
from concourse.bass_utils import run_bass_kernel_spmd

F32 = mybir.dt.float32
BF16 = mybir.dt.bfloat16
BF16_NP = mybir.dt.np(mybir.dt.bfloat16)

C = 16          # channels
HID = 128       # hidden dim
PW = 128        # partitions used as w-position within a half
N_CORES = 8
FIRE_RATE = 0.5
ALIVE_THR = 0.1

LAST_RESULTS = None  # BassKernelResults of the most recent kernel() call


# ---------------------------------------------------------------------------
# device program
# ---------------------------------------------------------------------------

def build_program(NI, H, W, TR=16):
    """Build the Bass program for one core processing NI images of HxW."""
    NH = W // PW                  # halves per row (2)
    assert W % PW == 0 and H % TR == 0 and TR == 16 and NH == 2
    Hp, Wp = H + 2, W + 2
    NRH = NI * H * NH             # total (img,row,half) count
    NT = NI * H // TR             # tiles

    nc = bacc.Bacc(trn_type="TRN2")

    # V: host-assembled perception operand. Sobel is separable, so the host
    # precomputes the vertical passes S = x[r-1]+2x[r]+x[r+1] and
    # D = x[r+1]-x[r-1]; the three horizontal taps of (S,D,x) are stacked on
    # 96 partitions with the +-1 column shifts baked into the layout.  fc0 is
    # then a SINGLE K=96 matmul per row-pair group (was 2: K=96 + K=48).
    vch = nc.dram_tensor("vch", [NI * (H // TR), 96, TR * Wp], BF16,
                         kind="ExternalInput")
    xpx_d = nc.dram_tensor("xpx", [PW, NRH, C], BF16, kind="ExternalInput")
    um_d = nc.dram_tensor("um", [PW, NRH], BF16, kind="ExternalInput")
    plx_d = nc.dram_tensor("plx", [PW, NRH], BF16, kind="ExternalInput")
    wf_d = nc.dram_tensor("wf", [96, HID], BF16, kind="ExternalInput")
    w1_d = nc.dram_tensor("w1d", [HID, C], BF16, kind="ExternalInput")
    se_d = nc.dram_tensor("SEd", [PW, PW], BF16, kind="ExternalInput")
    sw_d = nc.dram_tensor("SWd", [PW, PW], BF16, kind="ExternalInput")
    # dx stored pixel-major: [st, ti, w(128), rh(32), c]
    dxo = nc.dram_tensor("dxo", [NT // 2, 2, PW, 2 * TR, C], BF16,
                         kind="ExternalOutput")
    xno = nc.dram_tensor("xno", [PW, NRH, C], BF16, kind="ExternalOutput")

    with tile.TileContext(nc) as tc:
        _emit(tc, locals())
    nc.compile()
    return nc


def _emit(tc, t):
    nc = tc.nc
    NI, H, W, TR = t["NI"], t["H"], t["W"], t["TR"]
    NH, Hp, Wp, NRH, NT = t["NH"], t["Hp"], t["Wp"], t["NRH"], t["NT"]
    vch, xpx_d, um_d, plx_d = t["vch"], t["xpx_d"], t["um_d"], t["plx_d"]
    wf_d, w1_d, se_d, sw_d = (
        t["wf_d"], t["w1_d"], t["se_d"], t["sw_d"])
    dxo, xno = t["dxo"], t["xno"]
    HNH = H * NH                  # per-image rh span (512)
    AL = mybir.AluOpType
    Relu = mybir.ActivationFunctionType.Relu

    from contextlib import ExitStack
    ctx = ExitStack()
    with ctx:
        singles = ctx.enter_context(tc.tile_pool(name="singles", bufs=1))
        xc_pool = ctx.enter_context(tc.tile_pool(name="xc", bufs=8))
        hs_pool = ctx.enter_context(tc.tile_pool(name="hs", bufs=10))
        xp_pool = ctx.enter_context(tc.tile_pool(name="xp", bufs=4))
        dxs_pool = ctx.enter_context(tc.tile_pool(name="dxs", bufs=4))
        dxm_pool = ctx.enter_context(tc.tile_pool(name="dxm", bufs=4))
        xns_pool = ctx.enter_context(tc.tile_pool(name="xns", bufs=3))
        ps_h = ctx.enter_context(tc.tile_pool(name="ps_h", bufs=6, space="PSUM"))
        ps_dx = ctx.enter_context(tc.tile_pool(name="ps_dx", bufs=2, space="PSUM"))

        # ---- constants / weights / masks (gpsimd queue: SWDGE descriptors
        # go out immediately, ahead of the bulk sync-queue traffic) ----
        wf_sb = singles.tile([96, HID], BF16)
        nc.gpsimd.dma_start(out=wf_sb, in_=wf_d.ap())
        w1_sb = singles.tile([HID, C], BF16)
        nc.gpsimd.dma_start(out=w1_sb, in_=w1_d.ap())
        se_sb = singles.tile([PW, PW], BF16)
        nc.sync.dma_start(out=se_sb, in_=se_d.ap())
        sw_sb = singles.tile([PW, PW], BF16)
        nc.sync.dma_start(out=sw_sb, in_=sw_d.ap())

        um_sb = singles.tile([PW, NRH], BF16)
        plx_sb = singles.tile([PW, NRH], BF16)

        # ---- residents / mask scratch ----
        # xmid padded with NH zero-rows per image on both sides so the
        # vertical 3-max needs no edge-case ops (alpha 0 < 0.1 threshold)
        IMS = HNH + 2 * NH            # per-image rh stride in xmidp
        xmidp = singles.tile([PW, NI * IMS, C], BF16)
        for img in range(NI):
            nc.vector.memset(xmidp[:, img * IMS:img * IMS + NH, :], 0.0)
            nc.vector.memset(
                xmidp[:, img * IMS + NH + HNH:(img + 1) * IMS, :], 0.0)

        def xmid(rh0, n):
            # rh0 in unpadded global coords -> padded view [PW, n, C]
            img = rh0 // HNH
            base = img * IMS + NH + (rh0 - img * HNH)
            return xmidp[:, base:base + n, :]
        vm = singles.tile([PW, HNH], BF16)
        vm2 = singles.tile([PW, HNH], BF16)
        m3 = singles.tile([PW, HNH], BF16)
        life = singles.tile([PW, HNH], BF16)
        seam = singles.tile([PW, 2 * H], BF16)
        nc.vector.memset(seam, 0.0)

        # ---- PE pre-sync dummy (touch the hot matmul operand once) ----
        scr = ps_dx.tile([PW, 2], F32, tag="dxc")
        nc.tensor.matmul(out=scr, lhsT=wf_sb, rhs=wf_sb[:, 0:2],
                         start=True, stop=True)

        def half_slice(tile_, p0, cnt, hf, r0, n):
            return tile_[p0:p0 + cnt, r0:r0 + n].rearrange(
                "p (r h) -> p r h", h=NH)[:, :, hf]

        def mask_chunk(img, ck):
            # maxpool/life/x_new for rows [32*ck, 32*ck+32) of image img
            S = img * HNH
            B = img * IMS + NH
            r0 = ck * 64                  # rh offset within image
            CH = 64

            def A(k):  # alpha of x_mid rows, padded, strided view
                return xmidp[:, B + r0 + k:B + r0 + k + CH, 3]

            # vertical 3-max over zero-padded rows
            nc.vector.tensor_tensor(
                out=vm[:, r0:r0 + CH], in0=A(0), in1=A(-NH), op=AL.max)
            nc.vector.tensor_tensor(
                out=vm2[:, r0:r0 + CH], in0=vm[:, r0:r0 + CH], in1=A(NH),
                op=AL.max)

            # horizontal 3-max: PE shift-permutation matmuls (bf16) + seams
            pse = ps_h.tile([PW, CH], F32, tag="psh")
            nc.tensor.matmul(out=pse, lhsT=se_sb, rhs=vm2[:, r0:r0 + CH],
                             start=True, stop=True)
            psw = ps_h.tile([PW, CH], F32, tag="psh")
            nc.tensor.matmul(out=psw, lhsT=sw_sb, rhs=vm2[:, r0:r0 + CH],
                             start=True, stop=True)
            nc.vector.tensor_tensor(out=m3[:, r0:r0 + CH],
                                    in0=vm2[:, r0:r0 + CH], in1=pse, op=AL.max)
            nc.vector.tensor_tensor(out=m3[:, r0:r0 + CH],
                                    in0=m3[:, r0:r0 + CH], in1=psw, op=AL.max)

            # seam fixes between the two w-halves (rows 32*ck..32*ck+31)
            c0 = ck * 32
            nc.sync.dma_start(
                out=seam[127:128, c0:c0 + 32],
                in_=vm2[0:1, r0:r0 + CH].rearrange(
                    "p (r h) -> p r h", h=NH)[:, :, 1])
            nc.vector.tensor_tensor(
                out=half_slice(m3, 96, 32, 0, r0, CH),
                in0=half_slice(m3, 96, 32, 0, r0, CH),
                in1=seam[96:128, c0:c0 + 32], op=AL.max)
            nc.sync.dma_start(
                out=seam[0:1, H + c0:H + c0 + 32],
                in_=vm2[127:128, r0:r0 + CH].rearrange(
                    "p (r h) -> p r h", h=NH)[:, :, 0])
            nc.vector.tensor_tensor(
                out=half_slice(m3, 0, 32, 1, r0, CH),
                in0=half_slice(m3, 0, 32, 1, r0, CH),
                in1=seam[0:32, H + c0:H + c0 + 32], op=AL.max)

            # life = (m3 > thr) * pre_life
            nc.vector.scalar_tensor_tensor(
                out=life[:, r0:r0 + CH], in0=m3[:, r0:r0 + CH],
                scalar=ALIVE_THR, in1=plx_sb[:, S + r0:S + r0 + CH],
                op0=AL.is_gt, op1=AL.mult)

            # x_new = x_mid * life
            xns = xns_pool.tile([PW, CH, C], BF16)
            nc.gpsimd.tensor_tensor(
                out=xns, in0=xmid(S + r0, CH),
                in1=life[:, r0:r0 + CH, None].to_broadcast([PW, CH, C]),
                op=AL.mult)
            nc.sync.dma_start(out=xno.ap()[:, S + r0:S + r0 + CH, :], in_=xns)

        # ================= main loop (2-tile super-tiles) =================
        NST = NT // 2

        def emit_loads(st):
            rh_s = st * 2 * TR * NH
            xp = xp_pool.tile([PW, 2 * TR * NH, C], BF16)
            nc.sync.dma_start(
                out=xp, in_=xpx_d.ap()[:, rh_s:rh_s + 2 * TR * NH, :])
            xcs = []
            for ti in range(2):
                tt = 2 * st + ti
                # host-assembled V tile: one fat DMA (96 partitions x
                # TR*Wp contiguous).  SWDGE spreads the per-partition
                # descriptors across all 16 SDMA engines.
                xc2 = xc_pool.tile([96, TR, Wp], BF16)
                nc.gpsimd.dma_start(
                    out=xc2.rearrange("p r w -> p (r w)"),
                    in_=vch.ap()[tt])
                xcs.append(xc2)
            return xp, xcs

        lds = [emit_loads(0), emit_loads(1), emit_loads(2)]
        nc.sync.dma_start(out=um_sb, in_=um_d.ap())
        nc.sync.dma_start(out=plx_sb, in_=plx_d.ap())
        for st in range(NST):
            xp, xcs = lds[st % 3]
            rh_s = st * 2 * TR * NH
            for ti in range(2):
                xc2 = xcs[ti]

                # ---- fc0 + fc1 woven per q so the Tensor FIFO never has a
                # long stretch of LDW-heavy 16-col fc1 matmuls (keeps the
                # HAM activity monitor seeing MM streaming in every window).
                # fc1 is h-STATIONARY (out = h_block.T @ w1): dx lands
                # pixel-major [128w, rh, C] directly in PSUM -> no garbage
                # lanes and no channel->pixel transpose anywhere. ----
                pshs = [None] * 8
                hsbs = [None] * 8
                pdpx = ps_dx.tile([PW, 2 * TR, C], F32, tag="dxc")
                for q in range(2):
                    for g in range(4):
                        pp = q * 4 + g
                        psh = ps_h.tile([HID, 512], F32, tag="psh")
                        pshs[pp] = psh
                        nc.tensor.matmul(
                            out=psh, lhsT=wf_sb,
                            rhs=xc2[0:96, 2 * pp:2 * pp + 2, 1:1 + W],
                            start=True, stop=True, skip_group_check=True)
                    for g in range(4):
                        pp = q * 4 + g
                        hsb = hs_pool.tile([HID, 512], BF16)
                        hsbs[pp] = hsb
                        if pp % 8 not in (3, 5, 7):
                            nc.scalar.activation(
                                out=hsb, in_=pshs[pp], func=Relu)
                        else:
                            nc.vector.tensor_scalar(
                                out=hsb, in0=pshs[pp], scalar1=0.0,
                                scalar2=None, op0=AL.max)
                for pp in range(8):
                    for b in range(4):
                        # hsb col block b=(r2,h): rh index = 4*pp+2*r2+h
                        nc.tensor.matmul(
                            out=pdpx[:, 4 * pp + b, :],
                            lhsT=hsbs[pp][:, 128 * b:128 * b + 128],
                            rhs=w1_sb, start=True, stop=True,
                            skip_group_check=True)

                # drain dx to SBUF (bf16, for the dx output) and apply the
                # update mask straight out of PSUM (DVE) for x_mid
                rh0 = rh_s + ti * 2 * TR
                umv = um_sb[:, rh0:rh0 + 2 * TR][:, :, None].to_broadcast(
                    [PW, 2 * TR, C])
                dxs = dxs_pool.tile([PW, 2 * TR, C], BF16)
                dxm = dxm_pool.tile([PW, 2 * TR, C], F32)
                nc.scalar.copy(out=dxs, in_=pdpx)
                nc.vector.tensor_tensor(out=dxm, in0=pdpx, in1=umv,
                                        op=AL.mult)
                nc.sync.dma_start(out=dxo.ap()[st, ti], in_=dxs)
                nc.gpsimd.tensor_tensor(
                    out=xmid(rh0, 2 * TR),
                    in0=xp[:, ti * 2 * TR:(ti + 1) * 2 * TR, :], in1=dxm,
                    op=AL.add)

            # prefetch three super-tiles ahead so the loads complete well
            # before fc0 needs them (the issue queue runs behind the tail)
            if st + 3 < NST:
                lds[st % 3] = emit_loads(st + 3)

            # mask chunks lag TWO super-tiles: chunk gc needs x_mid rows
            # through the first row of chunk gc+1 (super-tile gc+1), whose
            # tail ran during iteration gc+1.  Emitting at gc+2 means the
            # pse/psw shift-matmuls never head-of-line-block the Tensor
            # FIFO waiting on the transpose->gpsimd tail chain.
            if st >= 2:
                mask_chunk(*divmod(st - 2, NST // NI))

        # drain the last two mask chunks
        for gc in range(max(NST - 2, 0), NST):
            mask_chunk(*divmod(gc, NST // NI))


# ---------------------------------------------------------------------------
# host side
# ---------------------------------------------------------------------------

def make_weights(w0, w1):
    """Separable-sobel fc0 weights: wf [96,128] for the V operand
    [S,D @ w-1 | x,D @ w | S,D @ w+1] (S,D are the vertical passes)."""
    w0 = np.asarray(w0, np.float32)          # [48, 128]
    W0x = w0[0::3]                           # [16, 128]
    Wgx = w0[1::3]
    Wgy = w0[2::3]
    wf = np.concatenate([
        -Wgx / 8, Wgy / 8,                   # tap w-1: S, D
        W0x, 2 * Wgy / 8,                    # tap w  : x, D
        Wgx / 8, Wgy / 8,                    # tap w+1: S, D
    ], axis=0).astype(BF16_NP)
    return wf, np.asarray(w1, BF16_NP)


def _maxpool3(a):
    """3x3 max pool, stride 1, -inf padding. a: [NI, H, W]."""
    p = np.pad(a, ((0, 0), (1, 1), (1, 1)), constant_values=-np.inf)
    v = np.maximum(np.maximum(p[:, :-2, :], p[:, 1:-1, :]), p[:, 2:, :])
    return np.maximum(np.maximum(v[:, :, :-2], v[:, :, 1:-1]), v[:, :, 2:])


def _to_rh(a, NI, H, NH):
    """[NI, H, W] -> [PW, NI*H*NH] pixel-slab layout."""
    return np.ascontiguousarray(
        a.reshape(NI, H, NH, PW).transpose(3, 0, 1, 2)).reshape(PW, NI * H * NH)


def host_inputs(x_core, ur_core, wf, w1, H, W, TR=16):
    """Build the per-core input map from [NI,H,W,C] x and [NI,H,W,1] rand."""
    NI = x_core.shape[0]
    NH = W // PW
    Hp, Wp = H + 2, W + 2
    NRH = NI * H * NH
    HT = H // TR

    # vertical sobel passes, channel-major [NI, C, H, W]
    xc = np.ascontiguousarray(x_core.transpose(0, 3, 1, 2)).astype(np.float32)
    zp = np.zeros_like(xc[:, :, :1])
    xu = np.concatenate([zp, xc[:, :, :-1]], axis=2)   # x[r-1]
    xd = np.concatenate([xc[:, :, 1:], zp], axis=2)    # x[r+1]
    S = xu + 2 * xc + xd
    D = xd - xu

    def tiles(a):  # [NI,C,H,W] -> [NI,HT,C,TR,W]
        return a.reshape(NI, C, HT, TR, W).transpose(0, 2, 1, 3, 4)

    V = np.zeros((NI, HT, 96, TR, Wp), np.float32)
    V[:, :, 0:16, :, 2:2 + W] = tiles(S)     # supplies S[w-1] at col 1+w
    V[:, :, 16:32, :, 2:2 + W] = tiles(D)
    V[:, :, 32:48, :, 1:1 + W] = tiles(xc)   # supplies x[w]
    V[:, :, 48:64, :, 1:1 + W] = tiles(D)
    V[:, :, 64:80, :, 0:0 + W] = tiles(S)    # supplies S[w+1]
    V[:, :, 80:96, :, 0:0 + W] = tiles(D)
    vch = np.ascontiguousarray(V).reshape(
        NI * HT, 96, TR * Wp).astype(BF16_NP)

    x_px = np.ascontiguousarray(
        x_core.reshape(NI, H, NH, PW, C).transpose(3, 0, 1, 2, 4)
    ).reshape(PW, NRH, C).astype(BF16_NP)

    um = (ur_core[..., 0] <= FIRE_RATE).astype(np.float32)
    um_p = _to_rh(um, NI, H, NH).astype(BF16_NP)

    pre = (_maxpool3(x_core[..., 3]) > ALIVE_THR).astype(np.float32)
    plx_p = _to_rh(pre, NI, H, NH).astype(BF16_NP)

    return {
        "vch": vch,
        "xpx": x_px,
        "um": um_p,
        "plx": plx_p,
        "wf": wf,
        "w1d": w1,
        "SEd": np.eye(PW, k=-1, dtype=np.float32).astype(BF16_NP),
        "SWd": np.eye(PW, k=1, dtype=np.float32).astype(BF16_NP),
    }


def unpack_xno(dev, NI, H, W):
    """[PW, NRH, C] device layout -> [NI, H, W, C] float32."""
    NH = W // PW
    return np.ascontiguousarray(
        dev.astype(np.float32).reshape(PW, NI, H, NH, C).transpose(1, 2, 3, 0, 4)
    ).reshape(NI, H, W, C)


def unpack_dxo(dev, NI, H, W, TR=16):
    """[NST, 2, PW, 2*TR, C] pixel-major dx -> [NI, H, W, C] float32."""
    NSI = H // TR // 2            # super-tiles per image
    NH = W // PW
    # [img, st, ti, w, row, h, c] -> [img, st, ti, row, h, w, c]
    d = dev.astype(np.float32).reshape(NI, NSI, 2, PW, TR, NH, C)
    d = d.transpose(0, 1, 2, 4, 5, 3, 6)
    return np.ascontiguousarray(d).reshape(NI, H, W, C)


@functools.lru_cache(maxsize=2)
def _cached_program(NI, H, W, TR):
    return build_program(NI, H, W, TR=TR)


def kernel(x, update_rand, w0, w1):
    x = np.asarray(x, np.float32)
    update_rand = np.asarray(update_rand, np.float32)
    B, H, W, _ = x.shape
    NI = B // N_CORES
    wf, w1f = make_weights(w0, w1)

    nc = _cached_program(NI, H, W, 16)
    in_maps = [
        host_inputs(x[i * NI:(i + 1) * NI], update_rand[i * NI:(i + 1) * NI],
                    wf, w1f, H, W)
        for i in range(N_CORES)
    ]
    res = run_bass_kernel_spmd(nc, in_maps, core_ids=list(range(N_CORES)))
    global LAST_RESULTS
    LAST_RESULTS = res
    x_new = np.concatenate(
        [unpack_xno(r["xno"], NI, H, W) for r in res.results], axis=0)
    dx = np.concatenate(
        [unpack_dxo(r["dxo"], NI, H, W) for r in res.results], axis=0)
    return x_new, dx

